# revision 12
# baseline (speedup 1.0000x reference)
"""MoE SwiGLU FFN (grouped GEMM) Trainium2 kernel.

Problem: E=32 experts, T=65536 tokens pre-sorted by expert (uniform 2048
tokens/expert), D=512, H=1024.
    h1 = ragged_dot(x, w1) + b1[seg]; h3 = ragged_dot(x, w3) + b3[seg]
    out = ragged_dot(silu(h1)*h3, w2) + b2[seg]

Sharding: expert parallelism across 8 cores. Tokens are pre-sorted and
uniformly dispatched, so expert-parallel == token-parallel: core c owns
experts [4c, 4c+4) and token rows [8192c, 8192(c+1)). No collectives.

Per-core kernel (all matmuls bf16 with fp32 PSUM accumulation; the trace
showed fp32r compiles to fp32_mode=HIGH 2-pass matmuls at ~284 ns/MM
effective, while bf16 streams 1 row/cycle ~216 ns/MM; bf16 end-to-end
error ~4e-3 passes the 2e-2 gate with margin):
  - x slab is passed host-transposed as xt [D, TPC] so every DMA is a
    contiguous-row load (the contraction over D needs D on partitions).
  - weights/x are cast to bf16 on the host and shipped as bf16 DRAM
    images: halves HBM read traffic (40->20 MB/core) and first-expert
    weight-load latency vs fp32+cast paths.
  - b1/b3 are host-pretransposed to [P, EPC*MH] (partition = H mod 128)
    and loaded in one contiguous DMA at start: the old per-expert
    rearrange gathers (4 B/packet) took ~19 us and stalled the first
    expert's PSUM eviction (8.4 us PE gap in the trace).
  - GEMM1/3 produce H^T tiles [H-part, token-free]; ACT applies
    silu(psum1+b1) (bias is per-partition in this layout), DVE fuses
    (psum3+b3)*silu -> bf16 in one scalar_tensor_tensor op; GEMM2 then
    contracts H on partitions with no further transposes, and DVE evicts
    psum2 + broadcast(b2) -> fp32 out tile, stored contiguously.
  - chunk-level software pipeline: GEMM2 of chunk i is emitted after
    GEMM1/3 of chunk i+1 so PE never waits on the SwiGLU tail.
"""

import numpy as np

import concourse.bass as bass
import concourse.mybir as mybir
from concourse.bass_utils import run_bass_kernel_spmd
from concourse.tile import TileContext

E, T, D, H = 32, 65536, 512, 1024
NCORES = 8
EPC = E // NCORES        # experts per core
TPC = T // NCORES        # tokens per core
TPE = T // E             # tokens per expert
NT = 512                 # token chunk (one PSUM bank in fp32)
P = 128

FP32 = mybir.dt.float32
BF16 = mybir.dt.bfloat16
DT_MM = BF16             # matmul operand dtype (1 row/cycle on the PE)
AF = mybir.ActivationFunctionType
ALU = mybir.AluOpType

KD = D // P              # 4 k-tiles for GEMM1/3
KH = H // P              # 8 k-tiles for GEMM2
MH = H // P              # 8 h m-tiles per chunk
NCHUNK = TPE // NT       # 4 chunks per expert
MT = NT // P             # 4 token sub-tiles per chunk


def _split_sync_waits(nc, max_waits=1):
    """The external neuronxcc walrus only accepts one sync-wait command per
    instruction; hoist excess waits onto preceding NoOps on the same engine."""
    n = 0
    for fn in nc.m.functions:
        for bb in fn.blocks:
            insts = bb.instructions
            i = 0
            while i < len(insts):
                inst = insts[i]
                si = inst.sync_info
                if si is not None and len(si.on_wait) > max_waits:
                    waits = list(si.on_wait)
                    while len(waits) > max_waits:
                        chunk, waits = waits[:max_waits], waits[max_waits:]
                        nop = mybir.InstNoOp(name=f"wait-split-{n}", ins=[], outs=[])
                        n += 1
                        nop.engine = inst.engine
                        nop.sync_info = mybir.SyncInfo(on_wait=chunk, on_update=[])
                        insts.insert(i, nop)
                        i += 1
                    inst.sync_info = mybir.SyncInfo(on_wait=waits, on_update=si.on_update)
                i += 1
    return n


def build_nc():
    nc = bass.Bass()

    xt = nc.declare_dram_parameter("xt", [D, TPC], DT_MM, isOutput=False)
    w1 = nc.declare_dram_parameter("w1", [EPC, D, H], DT_MM, isOutput=False)
    w3 = nc.declare_dram_parameter("w3", [EPC, D, H], DT_MM, isOutput=False)
    w2 = nc.declare_dram_parameter("w2", [EPC, H, D], DT_MM, isOutput=False)
    # b1/b3 host-pretransposed: [p, e*MH+m] = b[e, m*128+p]
    b1t = nc.declare_dram_parameter("b1t", [P, EPC * MH], FP32, isOutput=False)
    b3t = nc.declare_dram_parameter("b3t", [P, EPC * MH], FP32, isOutput=False)
    b2 = nc.declare_dram_parameter("b2", [EPC, D], FP32, isOutput=False)
    out = nc.declare_dram_parameter("out", [TPC, D], FP32, isOutput=True)

    with TileContext(nc) as tc:
        with (
            tc.tile_pool(name="w1p", bufs=2 * KD) as w1pool,
            tc.tile_pool(name="w3p", bufs=2 * KD) as w3pool,
            tc.tile_pool(name="w2p", bufs=2 * KH) as w2pool,
            tc.tile_pool(name="bias", bufs=2) as bias_pool,
            tc.tile_pool(name="xp", bufs=3 * KD) as xpool,
            tc.tile_pool(name="hp", bufs=2 * MH) as hpool,
            tc.tile_pool(name="t1p", bufs=9) as t1pool,
            tc.tile_pool(name="op", bufs=4) as opool,
            tc.tile_pool(name="ps13", bufs=6, space="PSUM") as ps13,
            tc.tile_pool(name="ps2", bufs=2, space="PSUM") as ps2,
            tc.tile_pool(name="const", bufs=1) as const_pool,
        ):
            ones_row = const_pool.tile([1, P], FP32, tag="ones", name="ones_row")
            nc.vector.memset(ones_row[:], 1.0)
            # PE p-state warmup: the tensor engine clock ramps with sustained
            # use (~3 us to max). Real matmuls can't start until the first
            # weight DMAs land (~11.6 us), so burn dependency-free matmuls on
            # a memset tile during that window; results are never read.
            wz = const_pool.tile([P, NT], DT_MM, tag="warm", name="warm_zeros")
            nc.vector.memset(wz[:], 0.0)
            # 8 MMs ~ 1.2 (cold) + 3x0.63 (mid) + 4x0.38 (max) ~ 4.6 us, which
            # fills [~6.3 us preamble end, ~11.6 us first-weight arrival].
            pwarm = ps2.tile([P, NT], FP32, tag="p2", name="p_warm")
            for i in range(8):
                nc.tensor.matmul(pwarm[:], lhsT=wz[:, :P], rhs=wz[:],
                                 start=True, stop=True)
            # all b1/b3 biases for this core's 4 experts: one contiguous DMA
            b1s = const_pool.tile([P, EPC * MH], FP32, tag="b1t", name="b1t_s")
            nc.sync.dma_start(out=b1s[:], in_=b1t[:, :])
            b3s = const_pool.tile([P, EPC * MH], FP32, tag="b3t", name="b3t_s")
            nc.sync.dma_start(out=b3s[:], in_=b3t[:, :])

            def load_x(e, c):
                t0 = e * TPE + c * NT
                xbf = []
                for k in range(KD):
                    t = xpool.tile([P, NT], DT_MM, tag="x", name=f"x_{e}_{c}_{k}")
                    nc.gpsimd.dma_start(out=t[:], in_=xt[k * P:(k + 1) * P, t0:t0 + NT])
                    xbf.append(t)
                return xbf

            def load_w13(e, first=False):
                """GEMM1/3 weights. For the first expert, interleave chunk-0 x
                tiles between w1 k-tiles so the m=0 matmuls can start as soon
                as possible. (Keep these on the gpsimd SWDGE: routing them via
                sync/scalar HWDGE queues measured ~8 us SLOWER to first MM —
                HWDGE tile loads don't fan out across the 16 DMA engines.)"""
                w1bf, w3bf = [], []
                x0 = []
                for k in range(KD):
                    t = w1pool.tile([P, H], DT_MM, tag="w1", name=f"w1_{e}_{k}")
                    nc.gpsimd.dma_start(out=t[:], in_=w1[e, k * P:(k + 1) * P, :])
                    w1bf.append(t)
                    if first:
                        xk = xpool.tile([P, NT], DT_MM, tag="x", name=f"x_{e}_0_{k}")
                        nc.gpsimd.dma_start(out=xk[:], in_=xt[k * P:(k + 1) * P, e * TPE:e * TPE + NT])
                        x0.append(xk)
                for k in range(KD):
                    t = w3pool.tile([P, H], DT_MM, tag="w3", name=f"w3_{e}_{k}")
                    nc.gpsimd.dma_start(out=t[:], in_=w3[e, k * P:(k + 1) * P, :])
                    w3bf.append(t)
                return dict(w1=w1bf, w3=w3bf, x0=x0)

            def load_w2b2(e, wts):
                w2bf = []
                for k in range(KH):
                    t = w2pool.tile([P, D], DT_MM, tag="w2", name=f"w2_{e}_{k}")
                    nc.gpsimd.dma_start(out=t[:], in_=w2[e, k * P:(k + 1) * P, :])
                    w2bf.append(t)
                b2r = bias_pool.tile([1, D], FP32, tag="b2r", name=f"b2r_{e}")
                nc.sync.dma_start(out=b2r[:], in_=b2[e][None, :])
                # broadcast b2 across partitions: ones[1,128].T @ b2r[1,512]
                b2p = ps2.tile([P, D], FP32, tag="p2", name=f"b2p_{e}")
                nc.tensor.matmul(b2p[:], lhsT=ones_row[:], rhs=b2r[:],
                                 start=True, stop=True)
                b2b = bias_pool.tile([P, D], FP32, tag="b2b", name=f"b2b_{e}")
                nc.scalar.copy(b2b[:], b2p[:])
                wts["w2"] = w2bf
                wts["b2"] = b2b

            def emit_gemm13(e, c, wts, xbf=None, split=False):
                """GEMM1+GEMM3+SwiGLU for chunk c of expert e -> 8 bf16 H^T tiles.

                split=True (first chunk only): run all p1 matmuls + silu before
                any p3 matmul, so the PE only needs w1+x to start and w3's DMA
                arrival is hidden behind the p1 pass."""
                if xbf is None:
                    xbf = load_x(e, c)
                htiles = []
                p1s, t1s = [], []
                for m in range(MH):
                    bcol = e * MH + m
                    p1 = ps13.tile([P, NT], FP32, tag="p13", name=f"p1_{e}_{c}_{m}")
                    for k in range(KD):
                        nc.tensor.matmul(
                            p1[:], lhsT=wts["w1"][k][:, m * P:(m + 1) * P], rhs=xbf[k][:],
                            start=(k == 0), stop=(k == KD - 1))
                    t1 = t1pool.tile([P, NT], FP32, tag="t1", name=f"t1_{e}_{c}_{m}")
                    nc.scalar.activation(t1[:], p1[:], AF.Silu,
                                         bias=b1s[:, bcol:bcol + 1], scale=1.0)
                    t1s.append(t1)
                    if split:
                        continue
                    p3 = ps13.tile([P, NT], FP32, tag="p13", name=f"p3_{e}_{c}_{m}")
                    for k in range(KD):
                        nc.tensor.matmul(
                            p3[:], lhsT=wts["w3"][k][:, m * P:(m + 1) * P], rhs=xbf[k][:],
                            start=(k == 0), stop=(k == KD - 1))
                    hbf = hpool.tile([P, NT], DT_MM, tag="h", name=f"h_{e}_{c}_{m}")
                    nc.vector.scalar_tensor_tensor(
                        out=hbf[:], in0=p3[:], scalar=b3s[:, bcol:bcol + 1], in1=t1[:],
                        op0=ALU.add, op1=ALU.mult)
                    htiles.append(hbf)
                if split:
                    for m in range(MH):
                        bcol = e * MH + m
                        p3 = ps13.tile([P, NT], FP32, tag="p13", name=f"p3_{e}_{c}_{m}")
                        for k in range(KD):
                            nc.tensor.matmul(
                                p3[:], lhsT=wts["w3"][k][:, m * P:(m + 1) * P], rhs=xbf[k][:],
                                start=(k == 0), stop=(k == KD - 1))
                        hbf = hpool.tile([P, NT], DT_MM, tag="h", name=f"h_{e}_{c}_{m}")
                        nc.vector.scalar_tensor_tensor(
                            out=hbf[:], in0=p3[:], scalar=b3s[:, bcol:bcol + 1],
                            in1=t1s[m], op0=ALU.add, op1=ALU.mult)
                        htiles.append(hbf)
                return htiles

            def emit_gemm2(e, c, wts, htiles):
                t0 = e * TPE + c * NT
                for mt in range(MT):
                    p2 = ps2.tile([P, D], FP32, tag="p2", name=f"p2_{e}_{c}_{mt}")
                    for k in range(KH):
                        nc.tensor.matmul(
                            p2[:], lhsT=htiles[k][:, mt * P:(mt + 1) * P],
                            rhs=wts["w2"][k][:],
                            start=(k == 0), stop=(k == KH - 1))
                    ot = opool.tile([P, D], FP32, tag="o", name=f"o_{e}_{c}_{mt}")
                    nc.vector.tensor_add(ot[:], p2[:], wts["b2"][:])
                    nc.sync.dma_start(
                        out=out[t0 + mt * P:t0 + (mt + 1) * P, :], in_=ot[:])

            # chunk-level pipeline across the whole (expert, chunk) sequence
            pending = None  # (e, c, wts, htiles)
            for e in range(EPC):
                wts_e = load_w13(e, first=(e == 0))
                for c in range(NCHUNK):
                    first = (e == 0 and c == 0)
                    h = emit_gemm13(e, c, wts_e,
                                    xbf=wts_e["x0"] if first else None,
                                    split=first)
                    if c == 0:
                        load_w2b2(e, wts_e)
                    if pending is not None:
                        emit_gemm2(*pending)
                    pending = (e, c, wts_e, h)
            emit_gemm2(*pending)

    _split_sync_waits(nc)
    return nc


_NC_CACHE = {}


def _get_nc():
    if "nc" not in _NC_CACHE:
        _NC_CACHE["nc"] = build_nc()
    return _NC_CACHE["nc"]


def prep_in_maps(x, w1, b1, w3, b3, w2, b2):
    """Per-core DRAM images (host-side sharding + layout + bf16 cast)."""
    bf16 = mybir.dt.np(DT_MM)
    in_maps = []
    for c in range(NCORES):
        es = slice(c * EPC, (c + 1) * EPC)
        # [p, e*MH+m] = b[e, m*128+p]
        b1t = np.ascontiguousarray(
            b1[es].reshape(EPC, MH, P).transpose(2, 0, 1).reshape(P, EPC * MH))
        b3t = np.ascontiguousarray(
            b3[es].reshape(EPC, MH, P).transpose(2, 0, 1).reshape(P, EPC * MH))
        in_maps.append(dict(
            xt=np.ascontiguousarray(x[c * TPC:(c + 1) * TPC].T).astype(bf16),
            w1=np.ascontiguousarray(w1[es]).astype(bf16),
            w3=np.ascontiguousarray(w3[es]).astype(bf16),
            w2=np.ascontiguousarray(w2[es]).astype(bf16),
            b1t=b1t.astype(np.float32), b3t=b3t.astype(np.float32),
            b2=np.ascontiguousarray(b2[es], dtype=np.float32),
        ))
    return in_maps


def _kernel_np_fallback(x, w1, b1, w3, b3, w2, b2, group_sizes):
    """Numpy reference path for non-uniform group sizes (not expected)."""
    bounds = np.cumsum(group_sizes)
    seg = np.searchsorted(bounds, np.arange(x.shape[0]), side="right")
    out = np.empty((x.shape[0], w2.shape[2]), np.float32)
    start = 0
    for e in range(len(group_sizes)):
        stop = start + int(group_sizes[e])
        xs = x[start:stop]
        h1 = xs @ w1[e] + b1[e]
        h3 = xs @ w3[e] + b3[e]
        h = (h1 / (1.0 + np.exp(-h1))) * h3
        out[start:stop] = h @ w2[e] + b2[e]
        start = stop
    return out


def kernel(x, w1, b1, w3, b3, w2, b2, group_sizes):
    gs = np.asarray(group_sizes)
    if not (gs.shape == (E,) and np.all(gs == T // E) and x.shape == (T, D)):
        return _kernel_np_fallback(np.asarray(x, np.float32), w1, b1, w3, b3,
                                   w2, b2, gs).astype(np.float32)

    x = np.asarray(x, np.float32)
    in_maps = prep_in_maps(x, np.asarray(w1, np.float32), np.asarray(b1, np.float32),
                           np.asarray(w3, np.float32), np.asarray(b3, np.float32),
                           np.asarray(w2, np.float32), np.asarray(b2, np.float32))
    nc = _get_nc()
    res = run_bass_kernel_spmd(nc, in_maps, list(range(NCORES)))
    return np.concatenate([res.results[c]["out"] for c in range(NCORES)], axis=0)


# revision 14
# speedup vs baseline: 1.1927x; 1.1927x over previous
"""MoE SwiGLU FFN (grouped GEMM) Trainium2 kernel.

Problem: E=32 experts, T=65536 tokens pre-sorted by expert (uniform 2048
tokens/expert), D=512, H=1024.
    h1 = ragged_dot(x, w1) + b1[seg]; h3 = ragged_dot(x, w3) + b3[seg]
    out = ragged_dot(silu(h1)*h3, w2) + b2[seg]

Sharding: expert parallelism across 8 cores. Tokens are pre-sorted and
uniformly dispatched, so expert-parallel == token-parallel: core c owns
experts [4c, 4c+4) and token rows [8192c, 8192(c+1)). No collectives.

Per-core kernel (all matmuls bf16 with fp32 PSUM accumulation; the trace
showed fp32r compiles to fp32_mode=HIGH 2-pass matmuls at ~284 ns/MM
effective, while bf16 streams 1 row/cycle ~216 ns/MM; bf16 end-to-end
error ~4e-3 passes the 2e-2 gate with margin):
  - x slab is passed host-transposed as xt [D, TPC] so every DMA is a
    contiguous-row load (the contraction over D needs D on partitions).
  - weights/x are cast to bf16 on the host and shipped as bf16 DRAM
    images: halves HBM read traffic (40->20 MB/core) and first-expert
    weight-load latency vs fp32+cast paths.
  - b1/b3 are host-pretransposed to [P, EPC*MH] (partition = H mod 128)
    and loaded in one contiguous DMA at start: the old per-expert
    rearrange gathers (4 B/packet) took ~19 us and stalled the first
    expert's PSUM eviction (8.4 us PE gap in the trace).
  - GEMM1/3 produce H^T tiles [H-part, token-free]; ACT applies
    silu(psum1+b1) (bias is per-partition in this layout), DVE fuses
    (psum3+b3)*silu -> bf16 in one scalar_tensor_tensor op; GEMM2 then
    contracts H on partitions with no further transposes, and DVE evicts
    psum2 + broadcast(b2) -> fp32 out tile, stored contiguously.
  - chunk-level software pipeline: GEMM2 of chunk i is emitted after
    GEMM1/3 of chunk i+1 so PE never waits on the SwiGLU tail.
"""

import numpy as np

import concourse.bass as bass
import concourse.mybir as mybir
from concourse.bass_utils import run_bass_kernel_spmd
from concourse.tile import TileContext

E, T, D, H = 32, 65536, 512, 1024
NCORES = 8
EPC = E // NCORES        # experts per core
TPC = T // NCORES        # tokens per core
TPE = T // E             # tokens per expert
NT = 512                 # token chunk (one PSUM bank in fp32)
P = 128

FP32 = mybir.dt.float32
BF16 = mybir.dt.bfloat16
DT_MM = BF16             # matmul operand dtype (1 row/cycle on the PE)
AF = mybir.ActivationFunctionType
ALU = mybir.AluOpType

KD = D // P              # 4 k-tiles for GEMM1/3
KH = H // P              # 8 k-tiles for GEMM2
MH = H // P              # 8 h m-tiles per chunk
NCHUNK = TPE // NT       # 4 chunks per expert
MT = NT // P             # 4 token sub-tiles per chunk


def _split_sync_waits(nc, max_waits=1):
    """The external neuronxcc walrus only accepts one sync-wait command per
    instruction; hoist excess waits onto preceding NoOps on the same engine."""
    n = 0
    for fn in nc.m.functions:
        for bb in fn.blocks:
            insts = bb.instructions
            i = 0
            while i < len(insts):
                inst = insts[i]
                si = inst.sync_info
                if si is not None and len(si.on_wait) > max_waits:
                    waits = list(si.on_wait)
                    while len(waits) > max_waits:
                        chunk, waits = waits[:max_waits], waits[max_waits:]
                        nop = mybir.InstNoOp(name=f"wait-split-{n}", ins=[], outs=[])
                        n += 1
                        nop.engine = inst.engine
                        nop.sync_info = mybir.SyncInfo(on_wait=chunk, on_update=[])
                        insts.insert(i, nop)
                        i += 1
                    inst.sync_info = mybir.SyncInfo(on_wait=waits, on_update=si.on_update)
                i += 1
    return n


def build_nc():
    nc = bass.Bass()

    xt = nc.declare_dram_parameter("xt", [D, TPC], DT_MM, isOutput=False)
    w1 = nc.declare_dram_parameter("w1", [EPC, D, H], DT_MM, isOutput=False)
    w3 = nc.declare_dram_parameter("w3", [EPC, D, H], DT_MM, isOutput=False)
    w2 = nc.declare_dram_parameter("w2", [EPC, H, D], DT_MM, isOutput=False)
    # b1/b3 host-pretransposed: [p, e*MH+m] = b[e, m*128+p]
    b1t = nc.declare_dram_parameter("b1t", [P, EPC * MH], FP32, isOutput=False)
    b3t = nc.declare_dram_parameter("b3t", [P, EPC * MH], FP32, isOutput=False)
    b2 = nc.declare_dram_parameter("b2", [EPC, D], FP32, isOutput=False)
    out = nc.declare_dram_parameter("out", [TPC, D], FP32, isOutput=True)

    with TileContext(nc) as tc:
        with (
            tc.tile_pool(name="w1p", bufs=2 * KD) as w1pool,
            tc.tile_pool(name="w3p", bufs=2 * KD) as w3pool,
            tc.tile_pool(name="w2p", bufs=2 * KH) as w2pool,
            tc.tile_pool(name="bias", bufs=2) as bias_pool,
            tc.tile_pool(name="xp", bufs=3 * KD) as xpool,
            tc.tile_pool(name="hp", bufs=2 * MH) as hpool,
            tc.tile_pool(name="t1p", bufs=9) as t1pool,
            tc.tile_pool(name="op", bufs=4) as opool,
            tc.tile_pool(name="ps13", bufs=6, space="PSUM") as ps13,
            tc.tile_pool(name="ps2", bufs=2, space="PSUM") as ps2,
            tc.tile_pool(name="const", bufs=1) as const_pool,
        ):
            ones_row = const_pool.tile([1, P], FP32, tag="ones", name="ones_row")
            nc.vector.memset(ones_row[:], 1.0)
            # PE p-state warmup: the tensor engine clock ramps with sustained
            # use (~3 us to max). Real matmuls can't start until the first
            # weight DMAs land (~11.6 us), so burn dependency-free matmuls on
            # a memset tile during that window; results are never read.
            # 8 MMs ~ 1.2 (cold) + 3x0.63 (mid) + 4x0.38 (max) ~ 4.6 us, which
            # fills [~6.3 us preamble end, ~11.6 us first-weight arrival].
            wz = const_pool.tile([P, NT], DT_MM, tag="warm", name="warm_zeros")
            nc.vector.memset(wz[:], 0.0)
            pwarm = ps2.tile([P, NT], FP32, tag="p2", name="p_warm")
            for i in range(8):
                nc.tensor.matmul(pwarm[:], lhsT=wz[:, :P], rhs=wz[:],
                                 start=True, stop=True)
            # all b1/b3 biases for this core's 4 experts: one contiguous DMA
            b1s = const_pool.tile([P, EPC * MH], FP32, tag="b1t", name="b1t_s")
            nc.sync.dma_start(out=b1s[:], in_=b1t[:, :])
            b3s = const_pool.tile([P, EPC * MH], FP32, tag="b3t", name="b3t_s")
            nc.sync.dma_start(out=b3s[:], in_=b3t[:, :])

            def load_x(e, c):
                t0 = e * TPE + c * NT
                xbf = []
                for k in range(KD):
                    t = xpool.tile([P, NT], DT_MM, tag="x", name=f"x_{e}_{c}_{k}")
                    nc.gpsimd.dma_start(out=t[:], in_=xt[k * P:(k + 1) * P, t0:t0 + NT])
                    xbf.append(t)
                return xbf

            def load_w13(e, first=False):
                """GEMM1/3 weights. For the first expert, interleave chunk-0 x
                tiles between w1 k-tiles so the m=0 matmuls can start as soon
                as possible. (Keep these on the gpsimd SWDGE: routing them via
                sync/scalar HWDGE queues measured ~8 us SLOWER to first MM —
                HWDGE tile loads don't fan out across the 16 DMA engines.)"""
                w1bf, w3bf = [], []
                x0 = []
                for k in range(KD):
                    t = w1pool.tile([P, H], DT_MM, tag="w1", name=f"w1_{e}_{k}")
                    nc.gpsimd.dma_start(out=t[:], in_=w1[e, k * P:(k + 1) * P, :])
                    w1bf.append(t)
                    if first:
                        xk = xpool.tile([P, NT], DT_MM, tag="x", name=f"x_{e}_0_{k}")
                        nc.gpsimd.dma_start(out=xk[:], in_=xt[k * P:(k + 1) * P, e * TPE:e * TPE + NT])
                        x0.append(xk)
                for k in range(KD):
                    t = w3pool.tile([P, H], DT_MM, tag="w3", name=f"w3_{e}_{k}")
                    nc.gpsimd.dma_start(out=t[:], in_=w3[e, k * P:(k + 1) * P, :])
                    w3bf.append(t)
                return dict(w1=w1bf, w3=w3bf, x0=x0)

            def load_w2b2(e, wts):
                w2bf = []
                for k in range(KH):
                    t = w2pool.tile([P, D], DT_MM, tag="w2", name=f"w2_{e}_{k}")
                    nc.gpsimd.dma_start(out=t[:], in_=w2[e, k * P:(k + 1) * P, :])
                    w2bf.append(t)
                b2r = bias_pool.tile([1, D], FP32, tag="b2r", name=f"b2r_{e}")
                nc.sync.dma_start(out=b2r[:], in_=b2[e][None, :])
                # broadcast b2 across partitions: ones[1,128].T @ b2r[1,512]
                b2p = ps2.tile([P, D], FP32, tag="p2", name=f"b2p_{e}")
                nc.tensor.matmul(b2p[:], lhsT=ones_row[:], rhs=b2r[:],
                                 start=True, stop=True)
                b2b = bias_pool.tile([P, D], FP32, tag="b2b", name=f"b2b_{e}")
                nc.scalar.copy(b2b[:], b2p[:])
                wts["w2"] = w2bf
                wts["b2"] = b2b

            def emit_gemm13(e, c, wts, xbf=None, split=False):
                """GEMM1+GEMM3+SwiGLU for chunk c of expert e -> 8 bf16 H^T tiles.

                split=True (first chunk only): run all p1 matmuls + silu before
                any p3 matmul, so the PE only needs w1+x to start and w3's DMA
                arrival is hidden behind the p1 pass."""
                if xbf is None:
                    xbf = load_x(e, c)
                htiles = []
                p1s, t1s = [], []
                for m in range(MH):
                    bcol = e * MH + m
                    p1 = ps13.tile([P, NT], FP32, tag="p13", name=f"p1_{e}_{c}_{m}")
                    for k in range(KD):
                        nc.tensor.matmul(
                            p1[:], lhsT=wts["w1"][k][:, m * P:(m + 1) * P], rhs=xbf[k][:],
                            start=(k == 0), stop=(k == KD - 1))
                    t1 = t1pool.tile([P, NT], FP32, tag="t1", name=f"t1_{e}_{c}_{m}")
                    nc.scalar.activation(t1[:], p1[:], AF.Silu,
                                         bias=b1s[:, bcol:bcol + 1], scale=1.0)
                    t1s.append(t1)
                    if split:
                        continue
                    p3 = ps13.tile([P, NT], FP32, tag="p13", name=f"p3_{e}_{c}_{m}")
                    for k in range(KD):
                        nc.tensor.matmul(
                            p3[:], lhsT=wts["w3"][k][:, m * P:(m + 1) * P], rhs=xbf[k][:],
                            start=(k == 0), stop=(k == KD - 1))
                    hbf = hpool.tile([P, NT], DT_MM, tag="h", name=f"h_{e}_{c}_{m}")
                    nc.vector.scalar_tensor_tensor(
                        out=hbf[:], in0=p3[:], scalar=b3s[:, bcol:bcol + 1], in1=t1[:],
                        op0=ALU.add, op1=ALU.mult)
                    htiles.append(hbf)
                if split:
                    for m in range(MH):
                        bcol = e * MH + m
                        p3 = ps13.tile([P, NT], FP32, tag="p13", name=f"p3_{e}_{c}_{m}")
                        for k in range(KD):
                            nc.tensor.matmul(
                                p3[:], lhsT=wts["w3"][k][:, m * P:(m + 1) * P], rhs=xbf[k][:],
                                start=(k == 0), stop=(k == KD - 1))
                        hbf = hpool.tile([P, NT], DT_MM, tag="h", name=f"h_{e}_{c}_{m}")
                        nc.vector.scalar_tensor_tensor(
                            out=hbf[:], in0=p3[:], scalar=b3s[:, bcol:bcol + 1],
                            in1=t1s[m], op0=ALU.add, op1=ALU.mult)
                        htiles.append(hbf)
                return htiles

            def emit_gemm2(e, c, wts, htiles):
                t0 = e * TPE + c * NT
                for mt in range(MT):
                    p2 = ps2.tile([P, D], FP32, tag="p2", name=f"p2_{e}_{c}_{mt}")
                    for k in range(KH):
                        nc.tensor.matmul(
                            p2[:], lhsT=htiles[k][:, mt * P:(mt + 1) * P],
                            rhs=wts["w2"][k][:],
                            start=(k == 0), stop=(k == KH - 1))
                    ot = opool.tile([P, D], FP32, tag="o", name=f"o_{e}_{c}_{mt}")
                    nc.vector.tensor_add(ot[:], p2[:], wts["b2"][:])
                    nc.sync.dma_start(
                        out=out[t0 + mt * P:t0 + (mt + 1) * P, :], in_=ot[:])

            # chunk-level pipeline across the whole (expert, chunk) sequence
            pending = None  # (e, c, wts, htiles)
            for e in range(EPC):
                wts_e = load_w13(e, first=(e == 0))
                for c in range(NCHUNK):
                    first = (e == 0 and c == 0)
                    h = emit_gemm13(e, c, wts_e,
                                    xbf=wts_e["x0"] if first else None,
                                    split=first)
                    if c == 0:
                        load_w2b2(e, wts_e)
                    if pending is not None:
                        emit_gemm2(*pending)
                    pending = (e, c, wts_e, h)
            emit_gemm2(*pending)

    _split_sync_waits(nc)
    return nc


_NC_CACHE = {}


def _get_nc():
    if "nc" not in _NC_CACHE:
        _NC_CACHE["nc"] = build_nc()
    return _NC_CACHE["nc"]


def prep_in_maps(x, w1, b1, w3, b3, w2, b2):
    """Per-core DRAM images (host-side sharding + layout + bf16 cast)."""
    bf16 = mybir.dt.np(DT_MM)
    in_maps = []
    for c in range(NCORES):
        es = slice(c * EPC, (c + 1) * EPC)
        # [p, e*MH+m] = b[e, m*128+p]
        b1t = np.ascontiguousarray(
            b1[es].reshape(EPC, MH, P).transpose(2, 0, 1).reshape(P, EPC * MH))
        b3t = np.ascontiguousarray(
            b3[es].reshape(EPC, MH, P).transpose(2, 0, 1).reshape(P, EPC * MH))
        in_maps.append(dict(
            xt=np.ascontiguousarray(x[c * TPC:(c + 1) * TPC].T).astype(bf16),
            w1=np.ascontiguousarray(w1[es]).astype(bf16),
            w3=np.ascontiguousarray(w3[es]).astype(bf16),
            w2=np.ascontiguousarray(w2[es]).astype(bf16),
            b1t=b1t.astype(np.float32), b3t=b3t.astype(np.float32),
            b2=np.ascontiguousarray(b2[es], dtype=np.float32),
        ))
    return in_maps


def _kernel_np_fallback(x, w1, b1, w3, b3, w2, b2, group_sizes):
    """Numpy reference path for non-uniform group sizes (not expected)."""
    bounds = np.cumsum(group_sizes)
    seg = np.searchsorted(bounds, np.arange(x.shape[0]), side="right")
    out = np.empty((x.shape[0], w2.shape[2]), np.float32)
    start = 0
    for e in range(len(group_sizes)):
        stop = start + int(group_sizes[e])
        xs = x[start:stop]
        h1 = xs @ w1[e] + b1[e]
        h3 = xs @ w3[e] + b3[e]
        h = (h1 / (1.0 + np.exp(-h1))) * h3
        out[start:stop] = h @ w2[e] + b2[e]
        start = stop
    return out


def kernel(x, w1, b1, w3, b3, w2, b2, group_sizes):
    gs = np.asarray(group_sizes)
    if not (gs.shape == (E,) and np.all(gs == T // E) and x.shape == (T, D)):
        return _kernel_np_fallback(np.asarray(x, np.float32), w1, b1, w3, b3,
                                   w2, b2, gs).astype(np.float32)

    x = np.asarray(x, np.float32)
    in_maps = prep_in_maps(x, np.asarray(w1, np.float32), np.asarray(b1, np.float32),
                           np.asarray(w3, np.float32), np.asarray(b3, np.float32),
                           np.asarray(w2, np.float32), np.asarray(b2, np.float32))
    nc = _get_nc()
    res = run_bass_kernel_spmd(nc, in_maps, list(range(NCORES)))
    return np.concatenate([res.results[c]["out"] for c in range(NCORES)], axis=0)
